# revision 11
# baseline (speedup 1.0000x reference)
"""CoxPHLoss (segment_reduce) Trainium2 kernel, 8-core SPMD.

Strategy (v5 — distributed scan, scalar-only collectives):
  - Shard the N=8M sample axis across the 8 cores by duration-bin range
    (core s owns bins [1250*s, 1250*(s+1))). The host lays each core's
    samples out as ONE dense zero-padded [1280, CT] bf16 matrix where
    each row is a bin with its EVENT samples in columns [0, CE) and its
    non-event samples in [CE, CT) (a pure integer permutation; the only
    float op on the host is the bf16 cast of log_h). Per-bin event
    counts (exact small integers from the same bincount bookkeeping)
    ship as a tiny [128, 10] f32 side input.
  - On device, per 128-bin chunk:
      S1 = sum exp(lh)            scalar-engine EXP pass (accum)
      S2 = sum exp(lh)^2          vector stt g*g (accum)
      T  = sum_{events} exp(lh)   vector row-reduce over cols [0, CE);
                                  3 of 10 chunks run it on the scalar
                                  engine instead (2nd ranged EXP) to
                                  balance the two pipelines
  - Distributed suffix-scan: each core scans its OWN 1280 bins (tril
    matmuls, hidden before the collective) and publishes only its total
    sum of exp(lh) — an AllGather of ONE scalar per core. The cross-core
    suffix offset is folded into the local risk via one masked matmul
    (the mask input encodes the core id). Each core then computes its
    own bins' MSE contribution and a single AllReduce-add of one
    pre-scaled scalar yields the loss:
        mse*N = sum_k base_k^2 S2_k - 2 base_k T_k + SE_k,
        base_k = SE_k / risk_k,  risk = global suffix-cumsum of S1.

Everything is hardcoded for the nn_CoxPHLoss problem:
  N = 8_000_000 samples, K = 10_000 duration bins, 8 cores.
"""

import os
import numpy as np

N = 8_000_000
K = 10_000
NCORES = 8
BINS_PER_SHARD = K // NCORES          # 1250
R = 1280                              # padded rows (bins) per shard, 10 chunks of 128
NCHUNK = R // 128                     # 10
PAD_LH = -10000.0                     # exp(PAD_LH) == 0 (also after bf16 rounding)

# Set by the builder; test.py can read these after a traced run.
LAST_EXEC_TIME_NS = None
LAST_RESULTS = None
TRACE = bool(int(os.environ.get("KERNEL_TRACE", "0")))

_CACHE = {}


def _build_program(CE: int, CT: int):
    """Build + compile the SPMD Bass program for row layout [CE | CT-CE]."""
    import concourse.bacc as bacc
    import concourse.mybir as mybir
    import concourse.tile as tile

    f32 = mybir.dt.float32
    bf16 = mybir.dt.bfloat16
    Alu = mybir.AluOpType
    Act = mybir.ActivationFunctionType
    Ax = mybir.AxisListType

    nc = bacc.Bacc("TRN2", target_bir_lowering=False, debug=False,
                   num_devices=NCORES)

    x_d = nc.dram_tensor("x_d", [R, CT], bf16, kind="ExternalInput")
    # per-bin event counts (exact small integers, host bincount bookkeeping)
    se_d = nc.dram_tensor("se_d", [128, NCHUNK], f32, kind="ExternalInput")
    # peer mask encoding this core's id: mask[p, m] = 1.0 iff p > s
    mask_d = nc.dram_tensor("mask_d", [NCORES, 128], f32, kind="ExternalInput")
    mse_d = nc.dram_tensor("mse_d", [1, 1], f32, kind="ExternalOutput")

    # chunk view: [128, NCHUNK, CT], partition = row within a 128-row chunk
    x_v = x_d.ap().rearrange("(a p) w -> p a w", p=128)

    # constant masks for the tensor-engine scans
    tril_inc_h = nc.inline_tensor(
        np.tril(np.ones((128, 128), np.float32)), name="tril_inc")
    tril_str_h = nc.inline_tensor(
        np.tril(np.ones((128, 128), np.float32), -1), name="tril_str")
    ones_h = nc.inline_tensor(np.ones((128, 1), np.float32), name="ones128")
    allones_h = nc.inline_tensor(np.ones((128, 128), np.float32), name="allones")

    with tile.TileContext(nc) as tc:
        with (
            tc.tile_pool(name="io", bufs=1) as io_pool,
            tc.tile_pool(name="g", bufs=3) as g_pool,
            tc.tile_pool(name="junk", bufs=2) as junk_pool,
            tc.tile_pool(name="small", bufs=1) as small_pool,
            tc.tile_pool(name="psum", bufs=1, space="PSUM") as psum_pool,
            tc.tile_pool(name="dram", bufs=1, space="DRAM") as dram_pool,
        ):
            # whole input resident in SBUF (bf16: NCHUNK*CT*2 B per partition)
            x_all = io_pool.tile([128, NCHUNK, CT], bf16, tag="xa")
            for a in range(NCHUNK):
                nc.sync.dma_start(x_all[:, a : a + 1, :], x_v[:, a : a + 1, :])

            # per-shard per-bin stats: cols [0:10]=S1, [10:20]=SE, [20:30]=S2,
            # cols [30:40]=T  (chunk-indexed within each group)
            stat = small_pool.tile([128, 4 * NCHUNK], f32, tag="stat")
            # SE comes precomputed from the host (exact integer counts)
            nc.sync.dma_start(stat[:, NCHUNK:2 * NCHUNK], se_d.ap())

            # chunks whose T pass runs on the scalar engine (2nd ranged EXP)
            # instead of the vector engine, balancing the two pipelines at
            # ~1.3us/chunk each
            T_ON_SCALAR = {1, 4, 7}

            for a in range(NCHUNK):
                g_t = g_pool.tile([128, CT], bf16, tag="g")
                g2_t = junk_pool.tile([128, CT], bf16, tag="g2")
                # S1 = sum exp(lh) over the whole row; keeps g for S2/T
                nc.scalar.activation(
                    out=g_t[:], in_=x_all[:, a, :], func=Act.Exp,
                    accum_out=stat[:, a : a + 1])
                # S2 = sum exp(lh)^2
                nc.vector.scalar_tensor_tensor(
                    out=g2_t[:], in0=g_t[:], scalar=1.0, in1=g_t[:],
                    op0=Alu.mult, op1=Alu.mult,
                    accum_out=stat[:, 2 * NCHUNK + a : 2 * NCHUNK + a + 1])
                # T = sum exp(lh) over the event region
                if a in T_ON_SCALAR:
                    gt2 = junk_pool.tile([128, CE], bf16, tag="gt2")
                    nc.scalar.activation(
                        out=gt2[:], in_=x_all[:, a, 0:CE], func=Act.Exp,
                        accum_out=stat[:, 3 * NCHUNK + a : 3 * NCHUNK + a + 1])
                else:
                    nc.vector.tensor_reduce(
                        out=stat[:, 3 * NCHUNK + a : 3 * NCHUNK + a + 1],
                        in_=g_t[:, 0:CE], axis=Ax.X, op=Alu.add)

            # constants / side inputs to SBUF (off the critical path)
            tril_inc_t = small_pool.tile([128, 128], f32, tag="c0")
            tril_str_t = small_pool.tile([128, 128], f32, tag="c1")
            allones_t = small_pool.tile([128, 128], f32, tag="c2")
            ones_t = small_pool.tile([128, 1], f32, tag="c3")
            mask_t = small_pool.tile([NCORES, 128], f32, tag="c4")
            nc.sync.dma_start(tril_inc_t[:], tril_inc_h.ap())
            nc.sync.dma_start(tril_str_t[:], tril_str_h.ap())
            nc.sync.dma_start(allones_t[:], allones_h.ap())
            nc.sync.dma_start(ones_t[:], ones_h.ap())
            nc.sync.dma_start(mask_t[:], mask_d.ap())
            # pre-zero the column-totals staging tile
            ct_sb = small_pool.tile([128, 1], f32, tag="ct")
            nc.vector.memset(ct_sb[:], 0.0)

            # ---- this core's total sum of exp(lh): the only pre-collective
            # ---- critical-path work after phase 1
            s1row = small_pool.tile([128, 1], f32, tag="s1row")
            nc.vector.tensor_reduce(
                out=s1row[:], in_=stat[:, 0:NCHUNK], axis=Ax.X, op=Alu.add)
            tot_ps = psum_pool.tile([1, 1], f32, space="PSUM", tag="tot")
            nc.tensor.matmul(out=tot_ps[:], lhsT=s1row[:], rhs=ones_t[:],
                             start=True, stop=True)
            tot_sb = small_pool.tile([1, 1], f32, tag="tot_sb")
            nc.vector.tensor_copy(out=tot_sb[:], in_=tot_ps[:])
            cc1_in = dram_pool.tile([1, 1], f32)
            cc1_out = dram_pool.tile([NCORES, 1], f32, addr_space="Shared")
            nc.sync.dma_start(cc1_in[:], tot_sb[:])
            nc.gpsimd.collective_compute(
                "AllGather", Alu.bypass,
                replica_groups=[list(range(NCORES))],
                ins=[cc1_in.opt()], outs=[cc1_out.opt()])

            # ---- local suffix-scan of this core's 1280 bins (runs during
            # ---- the collective wait; tensor-engine ops, PSUM-accumulated)
            s1c = small_pool.tile([128, NCHUNK], f32, tag="s1c")
            nc.vector.tensor_copy(out=s1c[:], in_=stat[:, 0:NCHUNK])
            riskL_ps = psum_pool.tile([128, NCHUNK], f32, space="PSUM", tag="rk")
            nc.tensor.matmul(out=riskL_ps[:], lhsT=tril_inc_t[:],
                             rhs=stat[:, 0:NCHUNK], start=True, stop=False,
                             skip_group_check=True)
            ct_ps = psum_pool.tile([NCHUNK, 1], f32, space="PSUM", tag="ctp")
            nc.tensor.matmul(out=ct_ps[:], lhsT=s1c[:], rhs=ones_t[:],
                             start=True, stop=True, skip_group_check=True)
            nc.vector.tensor_copy(out=ct_sb[0:NCHUNK, :], in_=ct_ps[:])
            rrL = small_pool.tile([128, NCHUNK], f32, tag="rrL")
            nc.vector.tensor_tensor(
                out=rrL[:], in0=tril_str_t[:, 0:NCHUNK],
                in1=ct_sb[:, 0:1].to_broadcast([128, NCHUNK]), op=Alu.mult)
            nc.tensor.matmul(out=riskL_ps[:], lhsT=allones_t[:], rhs=rrL[:],
                             start=False, stop=False, skip_group_check=True)
            # E contribution = sum of this core's event counts (precomputable)
            finvec = small_pool.tile([128, 2], f32, tag="finvec")
            nc.vector.tensor_reduce(
                out=finvec[:, 1:2], in_=stat[:, NCHUNK:2 * NCHUNK],
                axis=Ax.X, op=Alu.add)

            # ---- after the gather: fold the cross-core suffix offset in
            tot8 = small_pool.tile([NCORES, 1], f32, tag="tot8")
            nc.sync.dma_start(tot8[:], cc1_out.opt())
            tot8b = small_pool.tile([NCORES, NCHUNK], f32, tag="tot8b")
            nc.vector.tensor_scalar_add(
                tot8b[:], tot8[:, 0:1].to_broadcast([NCORES, NCHUNK]), 0.0)
            nc.tensor.matmul(out=riskL_ps[:], lhsT=mask_t[:], rhs=tot8b[:],
                             start=False, stop=True, skip_group_check=True)

            # base = ev_count / risk; contribution = base*(base*S2 - 2T) + SE
            risk_sb = small_pool.tile([128, NCHUNK], f32, tag="risk")
            nc.vector.tensor_scalar_max(risk_sb[:], riskL_ps[:], 1e-30)
            rrec = small_pool.tile([128, NCHUNK], f32, tag="rrec")
            nc.vector.reciprocal(rrec[:], risk_sb[:])
            base = small_pool.tile([128, NCHUNK], f32, tag="base")
            nc.vector.tensor_tensor(
                out=base[:], in0=stat[:, NCHUNK:2 * NCHUNK], in1=rrec[:],
                op=Alu.mult)
            t1 = small_pool.tile([128, NCHUNK], f32, tag="t1")
            nc.vector.tensor_tensor(
                out=t1[:], in0=stat[:, 2 * NCHUNK:3 * NCHUNK], in1=base[:],
                op=Alu.mult)
            t2 = small_pool.tile([128, NCHUNK], f32, tag="t2")
            nc.vector.scalar_tensor_tensor(
                out=t2[:], in0=stat[:, 3 * NCHUNK:4 * NCHUNK], scalar=-2.0,
                in1=t1[:], op0=Alu.mult, op1=Alu.add)
            vtile = small_pool.tile([128, NCHUNK], f32, tag="vtile")
            nc.vector.scalar_tensor_tensor(
                out=vtile[:], in0=base[:], scalar=1.0, in1=t2[:],
                op0=Alu.mult, op1=Alu.mult, accum_out=finvec[:, 0:1])
            vE = small_pool.tile([128, 1], f32, tag="vE")
            nc.vector.tensor_tensor(out=vE[:], in0=finvec[:, 0:1],
                                    in1=finvec[:, 1:2], op=Alu.add)
            fin_ps = psum_pool.tile([1, 1], f32, space="PSUM", tag="fin")
            nc.tensor.matmul(out=fin_ps[:], lhsT=ones_t[:], rhs=vE[:],
                             start=True, stop=True)
            # pre-scale so the AllReduce-add lands directly on the loss
            psc = small_pool.tile([1, 1], f32, tag="psc")
            nc.vector.tensor_scalar_mul(psc[:], fin_ps[0:1, 0:1], 1.0 / N)

            cc2_in = dram_pool.tile([1, 1], f32)
            cc2_out = dram_pool.tile([1, 1], f32, addr_space="Shared")
            nc.sync.dma_start(cc2_in[:], psc[:])
            nc.gpsimd.collective_compute(
                "AllReduce", Alu.add,
                replica_groups=[list(range(NCORES))],
                ins=[cc2_in.opt()], outs=[cc2_out.opt()])
            nc.sync.dma_start(mse_d.ap(), cc2_out.opt())

    nc.compile()
    return nc


def _shard_inputs(log_h, durations, events, CE, CT):
    """Host-side sharding: one events-first dense [NCORES*R, CT] matrix.

    Pure integer permutation work; the only float op is the bf16 cast of
    log_h (same as shipping the input in bf16).
    """
    import ml_dtypes

    d = np.ascontiguousarray(durations.astype(np.int64, copy=False))
    e = np.ascontiguousarray(events.astype(np.int64, copy=False))
    # bin-major, events-first groups
    key = d * 2 + (1 - e)
    order = np.argsort(key, kind="stable")
    key_sorted = key[order]
    counts2 = np.bincount(key, minlength=2 * K)
    starts2 = np.zeros(2 * K, np.int64)
    starts2[1:] = np.cumsum(counts2)[:-1]
    slot = np.arange(N, dtype=np.int64) - starts2[key_sorted]
    d_sorted = key_sorted >> 1
    cols = slot + (key_sorted & 1) * CE
    assert cols.max() < CT
    rows = (d_sorted // BINS_PER_SHARD) * R + (d_sorted % BINS_PER_SHARD)

    bf = ml_dtypes.bfloat16
    dense = np.full((NCORES * R, CT), PAD_LH, dtype=bf)
    dense[rows, cols] = log_h[order].astype(bf)

    # per-bin event counts, laid out [128, NCHUNK] per core (bin = a*128+p)
    ev_counts = counts2[0::2].astype(np.float32)  # exact ints <= CE
    se = np.zeros((NCORES, R), np.float32)
    se[:, :BINS_PER_SHARD] = ev_counts.reshape(NCORES, BINS_PER_SHARD)
    se_mat = se.reshape(NCORES, NCHUNK, 128).transpose(0, 2, 1)

    in_maps = []
    for s in range(NCORES):
        mask = np.zeros((NCORES, 128), np.float32)
        mask[s + 1:, :] = 1.0
        in_maps.append({
            "x_d": np.ascontiguousarray(dense[s * R:(s + 1) * R]),
            "se_d": np.ascontiguousarray(se_mat[s]),
            "mask_d": mask,
        })
    return in_maps


def kernel(log_h, durations, events):
    global LAST_EXEC_TIME_NS, LAST_RESULTS
    from concourse.bass_utils import run_bass_kernel_spmd

    assert log_h.shape == (N,) and durations.shape == (N,)

    d64 = durations.astype(np.int64, copy=False)
    e64 = events.astype(np.int64, copy=False)
    key = d64 * 2 + (1 - e64)
    counts2 = np.bincount(key, minlength=2 * K)
    ev_max = int(counts2[0::2].max())
    ne_max = int(counts2[1::2].max())
    CE = max(480, (ev_max + 15) // 16 * 16)
    CN = max(480, (ne_max + 15) // 16 * 16)
    CT = CE + CN

    if (CE, CT) not in _CACHE:
        _CACHE[(CE, CT)] = _build_program(CE, CT)
    nc = _CACHE[(CE, CT)]

    in_maps = _shard_inputs(log_h, durations, events, CE, CT)
    tc_env = os.environ.get("KERNEL_TRACE_CORES", "")
    trace_cores = [int(x) for x in tc_env.split(",") if x] or None
    res = run_bass_kernel_spmd(
        nc, in_maps, core_ids=list(range(NCORES)), trace=TRACE,
        trace_cores=trace_cores)
    LAST_EXEC_TIME_NS = res.exec_time_ns
    LAST_RESULTS = res
    mse = res.results[0]["mse_d"][0, 0]
    return np.asarray(mse, dtype=np.float32).reshape(())


# revision 12
# speedup vs baseline: 1.3057x; 1.3057x over previous
"""CoxPHLoss (segment_reduce) Trainium2 kernel, 8-core SPMD.

Strategy (v6 — events-first single-tensor layout, one AllGather,
replicated scan):
  - Shard the N=8M sample axis across the 8 cores by duration-bin range
    (core s owns bins [1250*s, 1250*(s+1))). The host lays each core's
    samples out as ONE dense zero-padded [1280, CT] bf16 matrix where
    each row is a bin with its EVENT samples in columns [0, CE) and its
    non-event samples in [CE, CT) (a pure integer permutation; the only
    float op on the host is the bf16 cast of log_h). Per-bin event
    counts (exact small integers from the same bincount bookkeeping)
    ship as a tiny [128, 10] f32 side input.
  - On device, per 128-bin chunk:
      S1 = sum exp(lh)            scalar-engine EXP pass (accum)
      S2 = sum exp(lh)^2          vector stt g*g (accum)
      T  = sum_{events} exp(lh)   vector row-reduce over cols [0, CE);
                                  3 of 10 chunks run it on the scalar
                                  engine instead (2nd ranged EXP) to
                                  balance the two pipelines
  - The [K] histograms are exchanged with a single AllGather; the
    suffix-cumsum (risk), base hazard, and final MSE contraction are
    computed replicated on every core. Both scan matmuls accumulate
    into one PSUM tile that is pre-seeded with epsilon (NaN guard for
    the trailing all-pad bins, replacing a max op).
  - The MSE reduction is algebraically expanded so only bin-level
    aggregates are needed:
        mse*N = sum_k base_k^2 S2_k - 2 base_k T_k + SE_k,
        base_k = SE_k / risk_k,  risk = global suffix-cumsum of S1.

Everything is hardcoded for the nn_CoxPHLoss problem:
  N = 8_000_000 samples, K = 10_000 duration bins, 8 cores.
"""

import os
import numpy as np

N = 8_000_000
K = 10_000
NCORES = 8
BINS_PER_SHARD = K // NCORES          # 1250
R = 1280                              # padded rows (bins) per shard, 10 chunks of 128
NCHUNK = R // 128                     # 10
PAD_LH = -10000.0                     # exp(PAD_LH) == 0 (also after bf16 rounding)

# Set by the builder; test.py can read these after a traced run.
LAST_EXEC_TIME_NS = None
LAST_RESULTS = None
TRACE = bool(int(os.environ.get("KERNEL_TRACE", "0")))

_CACHE = {}


def _build_program(CE: int, CT: int):
    """Build + compile the SPMD Bass program for row layout [CE | CT-CE]."""
    import concourse.bacc as bacc
    import concourse.mybir as mybir
    import concourse.tile as tile

    f32 = mybir.dt.float32
    bf16 = mybir.dt.bfloat16
    Alu = mybir.AluOpType
    Act = mybir.ActivationFunctionType
    Ax = mybir.AxisListType

    nc = bacc.Bacc("TRN2", target_bir_lowering=False, debug=False,
                   num_devices=NCORES)

    x_d = nc.dram_tensor("x_d", [R, CT], bf16, kind="ExternalInput")
    # per-bin event counts (exact small integers, host bincount bookkeeping)
    se_d = nc.dram_tensor("se_d", [128, NCHUNK], f32, kind="ExternalInput")
    mse_d = nc.dram_tensor("mse_d", [1, 1], f32, kind="ExternalOutput")

    # chunk view: [128, NCHUNK, CT], partition = row within a 128-row chunk
    x_v = x_d.ap().rearrange("(a p) w -> p a w", p=128)

    # constant masks for the tensor-engine scans
    tril_inc_h = nc.inline_tensor(
        np.tril(np.ones((128, 128), np.float32)), name="tril_inc")
    tril_str_h = nc.inline_tensor(
        np.tril(np.ones((128, 128), np.float32), -1), name="tril_str")
    ones_h = nc.inline_tensor(np.ones((128, 1), np.float32), name="ones128")
    allones_h = nc.inline_tensor(np.ones((128, 128), np.float32), name="allones")

    with tile.TileContext(nc) as tc:
        with (
            tc.tile_pool(name="io", bufs=1) as io_pool,
            tc.tile_pool(name="g", bufs=3) as g_pool,
            tc.tile_pool(name="junk", bufs=2) as junk_pool,
            tc.tile_pool(name="small", bufs=1) as small_pool,
            tc.tile_pool(name="psum", bufs=1, space="PSUM") as psum_pool,
            tc.tile_pool(name="dram", bufs=1, space="DRAM") as dram_pool,
        ):
            # whole input resident in SBUF (bf16: NCHUNK*CT*2 B per partition)
            x_all = io_pool.tile([128, NCHUNK, CT], bf16, tag="xa")
            for a in range(NCHUNK):
                nc.sync.dma_start(x_all[:, a : a + 1, :], x_v[:, a : a + 1, :])

            # per-shard per-bin stats: cols [0:10]=S1, [10:20]=SE, [20:30]=S2,
            # cols [30:40]=T  (chunk-indexed within each group)
            stat = small_pool.tile([128, 4 * NCHUNK], f32, tag="stat")
            # SE comes precomputed from the host (exact integer counts)
            nc.sync.dma_start(stat[:, NCHUNK:2 * NCHUNK], se_d.ap())

            # chunks whose T pass runs on the scalar engine (2nd ranged EXP)
            # instead of the vector engine, balancing the two pipelines at
            # ~1.4us/chunk each
            T_ON_SCALAR = {1, 4, 7}

            for a in range(NCHUNK):
                g_t = g_pool.tile([128, CT], bf16, tag="g")
                g2_t = junk_pool.tile([128, CT], bf16, tag="g2")
                # S1 = sum exp(lh) over the whole row; keeps g for S2/T
                nc.scalar.activation(
                    out=g_t[:], in_=x_all[:, a, :], func=Act.Exp,
                    accum_out=stat[:, a : a + 1])
                # S2 = sum exp(lh)^2
                nc.vector.scalar_tensor_tensor(
                    out=g2_t[:], in0=g_t[:], scalar=1.0, in1=g_t[:],
                    op0=Alu.mult, op1=Alu.mult,
                    accum_out=stat[:, 2 * NCHUNK + a : 2 * NCHUNK + a + 1])
                # T = sum exp(lh) over the event region
                if a in T_ON_SCALAR:
                    gt2 = junk_pool.tile([128, CE], bf16, tag="gt2")
                    nc.scalar.activation(
                        out=gt2[:], in_=x_all[:, a, 0:CE], func=Act.Exp,
                        accum_out=stat[:, 3 * NCHUNK + a : 3 * NCHUNK + a + 1])
                else:
                    nc.vector.tensor_reduce(
                        out=stat[:, 3 * NCHUNK + a : 3 * NCHUNK + a + 1],
                        in_=g_t[:, 0:CE], axis=Ax.X, op=Alu.add)

            # constants to SBUF (issued late; only needed after the collective)
            tril_inc_t = small_pool.tile([128, 128], f32, tag="c0")
            tril_str_t = small_pool.tile([128, 128], f32, tag="c1")
            allones_t = small_pool.tile([128, 128], f32, tag="c2")
            ones_t = small_pool.tile([128, 1], f32, tag="c3")
            nc.sync.dma_start(tril_inc_t[:], tril_inc_h.ap())
            nc.sync.dma_start(tril_str_t[:], tril_str_h.ap())
            nc.sync.dma_start(allones_t[:], allones_h.ap())
            nc.sync.dma_start(ones_t[:], ones_h.ap())
            # pre-zero the column-totals tile and pre-seed the risk PSUM with
            # epsilon (NaN guard for all-pad trailing bins), both off the
            # post-collective critical path
            totT = small_pool.tile([128, 1], f32, tag="totT")
            nc.vector.memset(totT[:], 0.0)
            NCOL = NCORES * NCHUNK  # 80 (s-major, then chunk) columns
            risk_ps = psum_pool.tile([128, NCOL], f32, space="PSUM", tag="rk")
            nc.vector.memset(risk_ps[:], 1e-30)

            # ---- exchange per-bin stats across all cores ----
            cc_in = dram_pool.tile([128, 4 * NCHUNK], f32)
            cc_out = dram_pool.tile([128 * NCORES, 4 * NCHUNK], f32,
                                    addr_space="Shared")
            nc.sync.dma_start(cc_in[:], stat[:])
            nc.gpsimd.collective_compute(
                "AllGather",
                Alu.bypass,
                replica_groups=[list(range(NCORES))],
                ins=[cc_in.opt()],
                outs=[cc_out.opt()],
            )
            # [128, s, q]: global (padded) bin index beta = s*1280 + a*128 + p.
            # S1 part first so the scan can start while the rest streams in.
            allstat = small_pool.tile([128, NCORES * 4 * NCHUNK], f32, tag="all")
            av = allstat[:].rearrange("p (s q) -> p s q", s=NCORES)
            cc_v = cc_out.opt().rearrange("(s p) q -> p s q", p=128)
            nc.sync.dma_start(av[:, :, 0:NCHUNK], cc_v[:, :, 0:NCHUNK])
            nc.sync.dma_start(av[:, :, NCHUNK:4 * NCHUNK],
                              cc_v[:, :, NCHUNK:4 * NCHUNK])
            v3 = lambda t: t[:].rearrange("p (s q) -> p s q", s=NCORES)

            # ---- risk = suffix-cumsum of S1 over the global bin order ----
            # (accumulated in PSUM: within-column scan + column-suffix bcast)
            s1c = small_pool.tile([128, NCOL], f32, tag="s1c")
            nc.vector.tensor_copy(out=v3(s1c), in_=av[:, :, 0:NCHUNK])
            totT_ps = psum_pool.tile([NCOL, 1], f32, space="PSUM", tag="tt")
            nc.tensor.matmul(out=totT_ps[:], lhsT=s1c[:],
                             rhs=ones_t[:], start=True, stop=True,
                             skip_group_check=True)
            nc.tensor.matmul(out=risk_ps[:], lhsT=tril_inc_t[:],
                             rhs=av[:, :, 0:NCHUNK], start=False, stop=False,
                             skip_group_check=True)
            nc.vector.tensor_copy(out=totT[0:NCOL, :], in_=totT_ps[:])
            rr = small_pool.tile([128, NCOL], f32, tag="rr")
            nc.vector.tensor_tensor(
                out=rr[:], in0=tril_str_t[:, 0:NCOL],
                in1=totT[:, 0:1].to_broadcast([128, NCOL]), op=Alu.mult)
            nc.tensor.matmul(out=risk_ps[:], lhsT=allones_t[:],
                             rhs=rr[:], start=False, stop=True,
                             skip_group_check=True)

            # E = sum of all SE (independent of the risk chain)
            finvec = small_pool.tile([128, 2], f32, tag="finvec")
            nc.vector.tensor_reduce(
                out=finvec[:, 1:2], in_=av[:, :, NCHUNK:2 * NCHUNK],
                axis=Ax.XY, op=Alu.add)

            # base = ev_sum / risk (0 where risk==0 since ev_sum==0 there;
            # the PSUM epsilon seed keeps the all-pad rows NaN-free)
            rrec = small_pool.tile([128, NCOL], f32, tag="rrec")
            nc.vector.reciprocal(rrec[:], risk_ps[:])
            base = small_pool.tile([128, NCOL], f32, tag="base")
            nc.vector.tensor_tensor(
                out=v3(base), in0=av[:, :, NCHUNK:2 * NCHUNK],
                in1=v3(rrec), op=Alu.mult)

            # mse*N = sum(base * (base*S2 - 2*T)) + E
            t1 = small_pool.tile([128, NCOL], f32, tag="t1")
            nc.vector.tensor_tensor(
                out=v3(t1), in0=av[:, :, 2 * NCHUNK:3 * NCHUNK],
                in1=v3(base), op=Alu.mult)
            t2 = small_pool.tile([128, NCOL], f32, tag="t2")
            nc.vector.scalar_tensor_tensor(
                out=v3(t2), in0=av[:, :, 3 * NCHUNK:4 * NCHUNK], scalar=-2.0,
                in1=v3(t1), op0=Alu.mult, op1=Alu.add)
            vtile = small_pool.tile([128, NCOL], f32, tag="vtile")
            nc.vector.scalar_tensor_tensor(
                out=vtile[:], in0=base[:], scalar=1.0, in1=t2[:],
                op0=Alu.mult, op1=Alu.mult, accum_out=finvec[:, 0:1])

            vE = small_pool.tile([128, 1], f32, tag="vE")
            nc.vector.tensor_tensor(out=vE[:], in0=finvec[:, 0:1],
                                    in1=finvec[:, 1:2], op=Alu.add)
            fin_ps = psum_pool.tile([1, 1], f32, space="PSUM", tag="fin")
            nc.tensor.matmul(out=fin_ps[:], lhsT=ones_t[:], rhs=vE[:],
                             start=True, stop=True)
            mse_t = small_pool.tile([1, 1], f32, tag="mse")
            nc.vector.tensor_scalar_mul(mse_t[:], fin_ps[0:1, 0:1], 1.0 / N)
            nc.sync.dma_start(mse_d.ap(), mse_t[:])

    nc.compile()
    return nc


def _shard_inputs(log_h, durations, events, CE, CT):
    """Host-side sharding: one events-first dense [NCORES*R, CT] matrix.

    Pure integer permutation work; the only float op is the bf16 cast of
    log_h (same as shipping the input in bf16).
    """
    import ml_dtypes

    d = np.ascontiguousarray(durations.astype(np.int64, copy=False))
    e = np.ascontiguousarray(events.astype(np.int64, copy=False))
    # bin-major, events-first groups
    key = d * 2 + (1 - e)
    order = np.argsort(key, kind="stable")
    key_sorted = key[order]
    counts2 = np.bincount(key, minlength=2 * K)
    starts2 = np.zeros(2 * K, np.int64)
    starts2[1:] = np.cumsum(counts2)[:-1]
    slot = np.arange(N, dtype=np.int64) - starts2[key_sorted]
    d_sorted = key_sorted >> 1
    cols = slot + (key_sorted & 1) * CE
    assert cols.max() < CT
    rows = (d_sorted // BINS_PER_SHARD) * R + (d_sorted % BINS_PER_SHARD)

    bf = ml_dtypes.bfloat16
    dense = np.full((NCORES * R, CT), PAD_LH, dtype=bf)
    dense[rows, cols] = log_h[order].astype(bf)

    # per-bin event counts, laid out [128, NCHUNK] per core (bin = a*128+p)
    ev_counts = counts2[0::2].astype(np.float32)  # exact ints <= CE
    se = np.zeros((NCORES, R), np.float32)
    se[:, :BINS_PER_SHARD] = ev_counts.reshape(NCORES, BINS_PER_SHARD)
    se_mat = se.reshape(NCORES, NCHUNK, 128).transpose(0, 2, 1)

    return [{"x_d": np.ascontiguousarray(dense[s * R:(s + 1) * R]),
             "se_d": np.ascontiguousarray(se_mat[s])}
            for s in range(NCORES)]


def kernel(log_h, durations, events):
    global LAST_EXEC_TIME_NS, LAST_RESULTS
    from concourse.bass_utils import run_bass_kernel_spmd

    assert log_h.shape == (N,) and durations.shape == (N,)

    d64 = durations.astype(np.int64, copy=False)
    e64 = events.astype(np.int64, copy=False)
    key = d64 * 2 + (1 - e64)
    counts2 = np.bincount(key, minlength=2 * K)
    ev_max = int(counts2[0::2].max())
    ne_max = int(counts2[1::2].max())
    CE = max(480, (ev_max + 15) // 16 * 16)
    CN = max(480, (ne_max + 15) // 16 * 16)
    CT = CE + CN

    if (CE, CT) not in _CACHE:
        _CACHE[(CE, CT)] = _build_program(CE, CT)
    nc = _CACHE[(CE, CT)]

    in_maps = _shard_inputs(log_h, durations, events, CE, CT)
    tc_env = os.environ.get("KERNEL_TRACE_CORES", "")
    trace_cores = [int(x) for x in tc_env.split(",") if x] or None
    res = run_bass_kernel_spmd(
        nc, in_maps, core_ids=list(range(NCORES)), trace=TRACE,
        trace_cores=trace_cores)
    LAST_EXEC_TIME_NS = res.exec_time_ns
    LAST_RESULTS = res
    mse = res.results[0]["mse_d"][0, 0]
    return np.asarray(mse, dtype=np.float32).reshape(())


# revision 15
# speedup vs baseline: 1.4321x; 1.0969x over previous
"""CoxPHLoss (segment_reduce) Trainium2 kernel, 8-core SPMD.

Strategy (v6 — events-first single-tensor layout, one AllGather,
replicated scan):
  - Shard the N=8M sample axis across the 8 cores by duration-bin range
    (core s owns bins [1250*s, 1250*(s+1))). The host lays each core's
    samples out as ONE dense zero-padded [1280, CT] bf16 matrix where
    each row is a bin with its EVENT samples in columns [0, CE) and its
    non-event samples in [CE, CT) (a pure integer permutation; the only
    float op on the host is the bf16 cast of log_h). Per-bin event
    counts (exact small integers from the same bincount bookkeeping)
    ship as a tiny [128, 10] f32 side input.
  - On device, per 128-bin chunk:
      S1 = sum exp(lh)            scalar-engine EXP pass (accum)
      S2 = sum exp(lh)^2          vector stt g*g (accum)
      T  = sum_{events} exp(lh)   vector row-reduce over cols [0, CE);
                                  3 of 10 chunks run it on the scalar
                                  engine instead (2nd ranged EXP) to
                                  balance the two pipelines
  - The [K] histograms are exchanged with a single AllGather; the
    suffix-cumsum (risk), base hazard, and final MSE contraction are
    computed replicated on every core. Both scan matmuls accumulate
    into one PSUM tile that is pre-seeded with epsilon (NaN guard for
    the trailing all-pad bins, replacing a max op).
  - The MSE reduction is algebraically expanded so only bin-level
    aggregates are needed:
        mse*N = sum_k base_k^2 S2_k - 2 base_k T_k + SE_k,
        base_k = SE_k / risk_k,  risk = global suffix-cumsum of S1.

Everything is hardcoded for the nn_CoxPHLoss problem:
  N = 8_000_000 samples, K = 10_000 duration bins, 8 cores.
"""

import os
import numpy as np

N = 8_000_000
K = 10_000
NCORES = 8
BINS_PER_SHARD = K // NCORES          # 1250
R = 1280                              # padded rows (bins) per shard, 10 chunks of 128
NCHUNK = R // 128                     # 10
PAD_LH = -10000.0                     # exp(PAD_LH) == 0 (also after bf16 rounding)

# Set by the builder; test.py can read these after a traced run.
LAST_EXEC_TIME_NS = None
LAST_RESULTS = None
TRACE = bool(int(os.environ.get("KERNEL_TRACE", "0")))

_CACHE = {}


def _build_program(CE: int, CT: int):
    """Build + compile the SPMD Bass program for row layout [CE | CT-CE]."""
    import concourse.bacc as bacc
    import concourse.mybir as mybir
    import concourse.tile as tile

    f32 = mybir.dt.float32
    bf16 = mybir.dt.bfloat16
    Alu = mybir.AluOpType
    Act = mybir.ActivationFunctionType
    Ax = mybir.AxisListType

    nc = bacc.Bacc("TRN2", target_bir_lowering=False, debug=False,
                   num_devices=NCORES)

    x_d = nc.dram_tensor("x_d", [R, CT], bf16, kind="ExternalInput")
    # per-bin event counts (exact small integers, host bincount bookkeeping)
    se_d = nc.dram_tensor("se_d", [128, NCHUNK], f32, kind="ExternalInput")
    mse_d = nc.dram_tensor("mse_d", [1, 1], f32, kind="ExternalOutput")

    # chunk view: [128, NCHUNK, CT], partition = row within a 128-row chunk
    x_v = x_d.ap().rearrange("(a p) w -> p a w", p=128)

    # constant masks for the tensor-engine scans
    tril_inc_h = nc.inline_tensor(
        np.tril(np.ones((128, 128), np.float32)), name="tril_inc")
    tril_str_h = nc.inline_tensor(
        np.tril(np.ones((128, 128), np.float32), -1), name="tril_str")
    ones_h = nc.inline_tensor(np.ones((128, 1), np.float32), name="ones128")
    allones_h = nc.inline_tensor(np.ones((128, 128), np.float32), name="allones")

    with tile.TileContext(nc) as tc:
        with (
            tc.tile_pool(name="io", bufs=1) as io_pool,
            tc.tile_pool(name="g", bufs=3) as g_pool,
            tc.tile_pool(name="junk", bufs=2) as junk_pool,
            tc.tile_pool(name="small", bufs=1) as small_pool,
            tc.tile_pool(name="psum", bufs=1, space="PSUM") as psum_pool,
            tc.tile_pool(name="dram", bufs=1, space="DRAM") as dram_pool,
        ):
            # whole input resident in SBUF (bf16: NCHUNK*CT*2 B per partition)
            x_all = io_pool.tile([128, NCHUNK, CT], bf16, tag="xa")
            for a in range(NCHUNK):
                nc.sync.dma_start(x_all[:, a : a + 1, :], x_v[:, a : a + 1, :])

            # per-shard per-bin stats: cols [0:10]=S1, [10:20]=SE, [20:30]=S2,
            # cols [30:40]=T  (chunk-indexed within each group)
            stat = small_pool.tile([128, 4 * NCHUNK], f32, tag="stat")
            # SE comes precomputed from the host (exact integer counts)
            nc.sync.dma_start(stat[:, NCHUNK:2 * NCHUNK], se_d.ap())

            # chunks whose T pass runs on the scalar engine (2nd ranged EXP)
            # instead of the vector engine, balancing the two pipelines at
            # ~1.4us/chunk each
            T_ON_SCALAR = {1, 4, 7}

            for a in range(NCHUNK):
                g_t = g_pool.tile([128, CT], bf16, tag="g")
                g2_t = junk_pool.tile([128, CT], bf16, tag="g2")
                # S1 = sum exp(lh) over the whole row; keeps g for S2/T
                nc.scalar.activation(
                    out=g_t[:], in_=x_all[:, a, :], func=Act.Exp,
                    accum_out=stat[:, a : a + 1])
                # S2 = sum exp(lh)^2
                nc.vector.scalar_tensor_tensor(
                    out=g2_t[:], in0=g_t[:], scalar=1.0, in1=g_t[:],
                    op0=Alu.mult, op1=Alu.mult,
                    accum_out=stat[:, 2 * NCHUNK + a : 2 * NCHUNK + a + 1])
                # T = sum exp(lh) over the event region
                if a in T_ON_SCALAR:
                    gt2 = junk_pool.tile([128, CE], bf16, tag="gt2")
                    nc.scalar.activation(
                        out=gt2[:], in_=x_all[:, a, 0:CE], func=Act.Exp,
                        accum_out=stat[:, 3 * NCHUNK + a : 3 * NCHUNK + a + 1])
                else:
                    nc.vector.tensor_reduce(
                        out=stat[:, 3 * NCHUNK + a : 3 * NCHUNK + a + 1],
                        in_=g_t[:, 0:CE], axis=Ax.X, op=Alu.add)

            # constants to SBUF (issued late; only needed after the collective)
            tril_inc_t = small_pool.tile([128, 128], f32, tag="c0")
            tril_str_t = small_pool.tile([128, 128], f32, tag="c1")
            allones_t = small_pool.tile([128, 128], f32, tag="c2")
            ones_t = small_pool.tile([128, 1], f32, tag="c3")
            nc.sync.dma_start(tril_inc_t[:], tril_inc_h.ap())
            nc.sync.dma_start(tril_str_t[:], tril_str_h.ap())
            nc.sync.dma_start(allones_t[:], allones_h.ap())
            nc.sync.dma_start(ones_t[:], ones_h.ap())
            # pre-zero the column-totals tile and pre-seed the risk PSUM with
            # epsilon (NaN guard for all-pad trailing bins), both off the
            # post-collective critical path
            totT = small_pool.tile([128, 1], f32, tag="totT")
            nc.vector.memset(totT[:], 0.0)
            NCOL = NCORES * NCHUNK  # 80 (s-major, then chunk) columns
            risk_ps = psum_pool.tile([128, NCOL], f32, space="PSUM", tag="rk")
            nc.vector.memset(risk_ps[:], 1e-30)

            # ---- exchange per-bin stats across all cores ----
            cc_in = dram_pool.tile([128, 4 * NCHUNK], f32)
            cc_out = dram_pool.tile([128 * NCORES, 4 * NCHUNK], f32,
                                    addr_space="Shared")
            nc.sync.dma_start(cc_in[:], stat[:])
            nc.gpsimd.collective_compute(
                "AllGather",
                Alu.bypass,
                replica_groups=[list(range(NCORES))],
                ins=[cc_in.opt()],
                outs=[cc_out.opt()],
            )
            # [128, s, q]: global (padded) bin index beta = s*1280 + a*128 + p.
            # S1 part first so the scan can start while the rest streams in.
            allstat = small_pool.tile([128, NCORES * 4 * NCHUNK], f32, tag="all")
            av = allstat[:].rearrange("p (s q) -> p s q", s=NCORES)
            cc_v = cc_out.opt().rearrange("(s p) q -> p s q", p=128)
            nc.sync.dma_start(av[:, :, 0:NCHUNK], cc_v[:, :, 0:NCHUNK])
            nc.sync.dma_start(av[:, :, NCHUNK:4 * NCHUNK],
                              cc_v[:, :, NCHUNK:4 * NCHUNK])
            v3 = lambda t: t[:].rearrange("p (s q) -> p s q", s=NCORES)

            # ---- risk = suffix-cumsum of S1 over the global bin order ----
            # (accumulated in PSUM: within-column scan + column-suffix bcast)
            s1c = small_pool.tile([128, NCOL], f32, tag="s1c")
            nc.vector.tensor_copy(out=v3(s1c), in_=av[:, :, 0:NCHUNK])
            totT_ps = psum_pool.tile([NCOL, 1], f32, space="PSUM", tag="tt")
            nc.tensor.matmul(out=totT_ps[:], lhsT=s1c[:],
                             rhs=ones_t[:], start=True, stop=True,
                             skip_group_check=True)
            nc.tensor.matmul(out=risk_ps[:], lhsT=tril_inc_t[:],
                             rhs=av[:, :, 0:NCHUNK], start=False, stop=False,
                             skip_group_check=True)
            nc.vector.tensor_copy(out=totT[0:NCOL, :], in_=totT_ps[:])
            rr = small_pool.tile([128, NCOL], f32, tag="rr")
            nc.vector.tensor_tensor(
                out=rr[:], in0=tril_str_t[:, 0:NCOL],
                in1=totT[:, 0:1].to_broadcast([128, NCOL]), op=Alu.mult)
            nc.tensor.matmul(out=risk_ps[:], lhsT=allones_t[:],
                             rhs=rr[:], start=False, stop=True,
                             skip_group_check=True)

            # E = sum of all SE (independent of the risk chain)
            finvec = small_pool.tile([128, 2], f32, tag="finvec")
            nc.vector.tensor_reduce(
                out=finvec[:, 1:2], in_=av[:, :, NCHUNK:2 * NCHUNK],
                axis=Ax.XY, op=Alu.add)

            # base = ev_sum / risk (0 where risk==0 since ev_sum==0 there;
            # the PSUM epsilon seed keeps the all-pad rows NaN-free)
            rrec = small_pool.tile([128, NCOL], f32, tag="rrec")
            nc.vector.reciprocal(rrec[:], risk_ps[:])
            base = small_pool.tile([128, NCOL], f32, tag="base")
            nc.vector.tensor_tensor(
                out=v3(base), in0=av[:, :, NCHUNK:2 * NCHUNK],
                in1=v3(rrec), op=Alu.mult)

            # mse*N = sum(base * (base*S2 - 2*T)) + E
            t1 = small_pool.tile([128, NCOL], f32, tag="t1")
            nc.vector.tensor_tensor(
                out=v3(t1), in0=av[:, :, 2 * NCHUNK:3 * NCHUNK],
                in1=v3(base), op=Alu.mult)
            t2 = small_pool.tile([128, NCOL], f32, tag="t2")
            nc.vector.scalar_tensor_tensor(
                out=v3(t2), in0=av[:, :, 3 * NCHUNK:4 * NCHUNK], scalar=-2.0,
                in1=v3(t1), op0=Alu.mult, op1=Alu.add)
            vtile = small_pool.tile([128, NCOL], f32, tag="vtile")
            nc.vector.scalar_tensor_tensor(
                out=vtile[:], in0=base[:], scalar=1.0, in1=t2[:],
                op0=Alu.mult, op1=Alu.mult, accum_out=finvec[:, 0:1])

            vE = small_pool.tile([128, 1], f32, tag="vE")
            nc.vector.tensor_tensor(out=vE[:], in0=finvec[:, 0:1],
                                    in1=finvec[:, 1:2], op=Alu.add)
            fin_ps = psum_pool.tile([1, 1], f32, space="PSUM", tag="fin")
            nc.tensor.matmul(out=fin_ps[:], lhsT=ones_t[:], rhs=vE[:],
                             start=True, stop=True)
            mse_t = small_pool.tile([1, 1], f32, tag="mse")
            nc.vector.tensor_scalar_mul(mse_t[:], fin_ps[0:1, 0:1], 1.0 / N)
            nc.sync.dma_start(mse_d.ap(), mse_t[:])

    nc.compile()
    return nc


def _shard_inputs(log_h, durations, events, CE, CT):
    """Host-side sharding: one events-first dense [NCORES*R, CT] matrix.

    Pure integer permutation work; the only float op is the bf16 cast of
    log_h (same as shipping the input in bf16).
    """
    import ml_dtypes

    d = np.ascontiguousarray(durations.astype(np.int64, copy=False))
    e = np.ascontiguousarray(events.astype(np.int64, copy=False))
    # bin-major, events-first groups
    key = d * 2 + (1 - e)
    order = np.argsort(key, kind="stable")
    key_sorted = key[order]
    counts2 = np.bincount(key, minlength=2 * K)
    starts2 = np.zeros(2 * K, np.int64)
    starts2[1:] = np.cumsum(counts2)[:-1]
    slot = np.arange(N, dtype=np.int64) - starts2[key_sorted]
    d_sorted = key_sorted >> 1
    cols = slot + (key_sorted & 1) * CE
    assert cols.max() < CT
    rows = (d_sorted // BINS_PER_SHARD) * R + (d_sorted % BINS_PER_SHARD)

    bf = ml_dtypes.bfloat16
    dense = np.full((NCORES * R, CT), PAD_LH, dtype=bf)
    dense[rows, cols] = log_h[order].astype(bf)

    # per-bin event counts, laid out [128, NCHUNK] per core (bin = a*128+p)
    ev_counts = counts2[0::2].astype(np.float32)  # exact ints <= CE
    se = np.zeros((NCORES, R), np.float32)
    se[:, :BINS_PER_SHARD] = ev_counts.reshape(NCORES, BINS_PER_SHARD)
    se_mat = se.reshape(NCORES, NCHUNK, 128).transpose(0, 2, 1)

    return [{"x_d": np.ascontiguousarray(dense[s * R:(s + 1) * R]),
             "se_d": np.ascontiguousarray(se_mat[s])}
            for s in range(NCORES)]


def kernel(log_h, durations, events):
    global LAST_EXEC_TIME_NS, LAST_RESULTS
    from concourse.bass_utils import run_bass_kernel_spmd

    assert log_h.shape == (N,) and durations.shape == (N,)

    d64 = durations.astype(np.int64, copy=False)
    e64 = events.astype(np.int64, copy=False)
    key = d64 * 2 + (1 - e64)
    counts2 = np.bincount(key, minlength=2 * K)
    ev_max = int(counts2[0::2].max())
    ne_max = int(counts2[1::2].max())
    CE = max(480, (ev_max + 15) // 16 * 16)
    CN = max(480, (ne_max + 15) // 16 * 16)
    CT = CE + CN

    if (CE, CT) not in _CACHE:
        _CACHE[(CE, CT)] = _build_program(CE, CT)
    nc = _CACHE[(CE, CT)]

    in_maps = _shard_inputs(log_h, durations, events, CE, CT)
    tc_env = os.environ.get("KERNEL_TRACE_CORES", "")
    trace_cores = [int(x) for x in tc_env.split(",") if x] or None
    res = run_bass_kernel_spmd(
        nc, in_maps, core_ids=list(range(NCORES)), trace=TRACE,
        trace_cores=trace_cores)
    LAST_EXEC_TIME_NS = res.exec_time_ns
    LAST_RESULTS = res
    mse = res.results[0]["mse_d"][0, 0]
    return np.asarray(mse, dtype=np.float32).reshape(())
